# revision 9
# baseline (speedup 1.0000x reference)
"""BailingMoE block (router + 16 routed experts top-4 + shared SwiGLU MLP)
as a Trainium2 Bass/Tile kernel, expert-parallel over 8 NeuronCores.

Sharding:
  - Routed expert weight stacks [E,H,I] split along E: 2 experts per core
    (cast to bf16 on host; fp32 PSUM accumulation on device).
  - Shared-expert MLP tensor-parallel along the intermediate dim: 128 of
    1024 shared-intermediate channels per core.
  - Router replicated (fp32 - top-4 selection must match the reference);
    per-core the router weight columns are permuted so that each core's own
    2 experts land in columns 0/1 (softmax/top-k are permutation invariant).
  - Each core produces a full [T,H] partial (its experts + its shared
    slice); two ReduceScatters (token halves) sum the partials on-device
    and leave each core with a disjoint 2x64-token slice of the output,
    which the host concatenates (pure unshard, no host math).

Device dataflow (per core, all matmuls bf16 with fp32 accumulation):
  Xt = X^T staged [H,T];  G_t/U_t = Wg^T X^T per expert in [I,T] layout so
  both operands of every matmul are in their native layout (no transposes
  on the heavy path);  H = silu(G)*U*combine;  down-proj accumulates both
  experts + shared slice into one PSUM tile per (token-tile, out-half).
"""

import numpy as np
import ml_dtypes

import concourse.bass as bass
import concourse.mybir as mybir
import concourse.tile as tile
from concourse import bacc
from concourse.bass_utils import run_bass_kernel_spmd
from concourse.masks import make_identity

BF16 = ml_dtypes.bfloat16

NCORES = 8
T = 1024
H = 1024
I = 512  # routed expert intermediate
E = 16
TOP_K = 4
E_LOC = 2  # experts per core
ISH = 128  # shared-intermediate channels per core (1024 / 8)
KT = H // 128  # 8 contraction tiles over H
NI = I // 128  # 4 partition tiles over I
NTT = T // 128  # 8 token tiles
NCHUNK = 4  # ReduceScatter chunks (token quarters)

F32 = mybir.dt.float32
BF = mybir.dt.bfloat16


def build_nc():
    nc = bacc.Bacc("TRN2", target_bir_lowering=False, debug=False,
                   num_devices=NCORES)

    xtf = nc.dram_tensor("xtf", [H, T], F32, kind="ExternalInput")
    xtb = nc.dram_tensor("xtb", [H, T], BF, kind="ExternalInput")
    rwt = nc.dram_tensor("rwt", [H, E], F32, kind="ExternalInput")
    wg = nc.dram_tensor("wg", [E_LOC, H, I], BF, kind="ExternalInput")
    wu = nc.dram_tensor("wu", [E_LOC, H, I], BF, kind="ExternalInput")
    wd = nc.dram_tensor("wd", [E_LOC, I, H], BF, kind="ExternalInput")
    wsgu = nc.dram_tensor("wsgu", [H, 2 * ISH], BF, kind="ExternalInput")
    wsd = nc.dram_tensor("wsd", [ISH, H], BF, kind="ExternalInput")
    o = nc.dram_tensor("o", [NCHUNK * (T // NCHUNK // NCORES), H], F32,
                       kind="ExternalOutput")

    rg = [list(range(NCORES))]

    with tile.TileContext(nc) as tc:
        with (
            tc.tile_pool(name="big", bufs=1) as big,
            tc.tile_pool(name="small", bufs=3) as small,
            tc.tile_pool(name="gs_pool", bufs=3) as gs_pool,
            tc.tile_pool(name="accs", bufs=3) as accs,
            tc.tile_pool(name="ps_small", bufs=2, space="PSUM") as ps_small,
            tc.tile_pool(name="ps_gu", bufs=2, space="PSUM") as ps_gu,
            tc.tile_pool(name="ps_acc", bufs=2, space="PSUM") as ps_acc,
            tc.tile_pool(name="dram", bufs=1, space="DRAM") as dram,
        ):
            # ---- staged inputs (everything fits in SBUF); DMAs chunked and
            # emitted in consumption order so compute starts early ----
            rwt_sb = big.tile([128, KT, E], F32)
            nc.sync.dma_start(out=rwt_sb, in_=rwt.rearrange("(k p) e -> p k e", p=128))
            xtf_r = xtf.rearrange("(k p) t -> p k t", p=128)
            xtf_sb = big.tile([128, KT, T], F32)
            for k in range(KT):
                nc.sync.dma_start(out=xtf_sb[:, k, :], in_=xtf_r[:, k, :])
            xtb_r = xtb.rearrange("(k p) t -> p k t", p=128)
            xtb_sb = big.tile([128, KT, T], BF)
            for k in range(KT):
                nc.sync.dma_start(out=xtb_sb[:, k, :], in_=xtb_r[:, k, :])
            wg_sb = big.tile([128, E_LOC, KT, I], BF)
            wu_sb = big.tile([128, E_LOC, KT, I], BF)
            wg_r = wg.rearrange("e (k p) i -> p e k i", p=128)
            wu_r = wu.rearrange("e (k p) i -> p e k i", p=128)
            for e in range(E_LOC):
                nc.sync.dma_start(out=wg_sb[:, e], in_=wg_r[:, e])
                nc.sync.dma_start(out=wu_sb[:, e], in_=wu_r[:, e])
            wsgu_sb = big.tile([128, KT, 2 * ISH], BF)
            nc.sync.dma_start(out=wsgu_sb, in_=wsgu.rearrange("(k p) i -> p k i", p=128))
            wd_sb = big.tile([128, E_LOC, NI, H], BF)
            nc.sync.dma_start(out=wd_sb, in_=wd.rearrange("e (n p) h -> p e n h", p=128))
            wsd_sb = big.tile([128, H], BF)
            nc.sync.dma_start(out=wsd_sb, in_=wsd[:])

            identity = big.tile([128, 128], F32)
            make_identity(nc, identity)

            # one DRAM tensor per chunk: a shared tensor would put a false
            # WAR dependency between chunk k's RS read and chunk k+1's writes
            acc_dram = [dram.tile([T // NCHUNK, H], F32, name=f"acc_dram{i}")
                        for i in range(NCHUNK)]
            rs_out = dram.tile([NCHUNK, T // NCHUNK // NCORES, H], F32)
            c_scr = dram.tile([E_LOC, T], BF)

            # ---- router: logits -> top-4 -> normalized combine weights.
            # All vector work batched over the 8 token tiles ([128, 8, 16]
            # layout, token-within-tile on partitions) to avoid 8 serial
            # cross-engine round-trips. ----
            L = small.tile([128, NTT, E], F32)
            for tt in range(NTT):
                ts = slice(tt * 128, (tt + 1) * 128)
                lg_ps = ps_small.tile([128, E], F32, tag="sm")
                for k in range(KT):
                    nc.tensor.matmul(lg_ps[:], xtf_sb[:, k, ts], rwt_sb[:, k, :],
                                     start=(k == 0), stop=(k == KT - 1))
                nc.vector.tensor_copy(L[:, tt, :], lg_ps[:])
            # 4 rounds of (rowmax, knock out the max) to find the 4th-largest
            m = small.tile([128, NTT, 4], F32)
            msk = small.tile([128, NTT, E], F32)
            Lw = small.tile([128, NTT, E], F32)
            nc.vector.tensor_copy(Lw[:], L[:])
            for r in range(4):
                nc.vector.reduce_max(m[:, :, r], Lw[:], axis=mybir.AxisListType.X)
                if r < 3:
                    nc.vector.tensor_tensor(msk[:], Lw[:],
                                            m[:, :, r:r + 1].to_broadcast([128, NTT, E]),
                                            op=mybir.AluOpType.is_ge)
                    nc.vector.scalar_tensor_tensor(Lw[:], msk[:], -1e30, Lw[:],
                                                   op0=mybir.AluOpType.mult,
                                                   op1=mybir.AluOpType.add)
            # top-4 mask on the original logits; softmax over the masked set
            nc.vector.tensor_tensor(msk[:], L[:],
                                    m[:, :, 3:4].to_broadcast([128, NTT, E]),
                                    op=mybir.AluOpType.is_ge)
            nc.vector.tensor_tensor(L[:], L[:],
                                    m[:, :, 0:1].to_broadcast([128, NTT, E]),
                                    op=mybir.AluOpType.subtract)
            nc.scalar.activation(L[:], L[:], mybir.ActivationFunctionType.Exp)
            nc.vector.tensor_mul(L[:], L[:], msk[:])
            ssum = small.tile([128, NTT, 1], F32)
            nc.vector.reduce_sum(ssum[:, :, 0], L[:], axis=mybir.AxisListType.X)
            rcp = small.tile([128, NTT, 1], F32)
            nc.vector.reciprocal(rcp[:, :, 0], ssum[:, :, 0])
            nc.vector.tensor_mul(L[:], L[:],
                                 rcp[:].to_broadcast([128, NTT, E]))
            # one PE transpose: [tok128, (tt e)] -> [(tt e), tok128]
            ct_ps = ps_small.tile([128, 128], F32, tag="sm")
            nc.tensor.transpose(ct_ps[:], L.rearrange("p t e -> p (t e)"),
                                identity[:])
            ct_sb = small.tile([128, 128], BF)
            nc.vector.tensor_copy(ct_sb[:], ct_ps[:])
            # per-expert combine rows broadcast to 128 partitions via a DRAM
            # round-trip (DMA replicates the row; frees PE/DVE)
            ct_v = ct_sb.rearrange("(t e) x -> t e x", e=E)
            c_sb = big.tile([128, E_LOC, T], BF)
            for e in range(E_LOC):
                nc.sync.dma_start(out=c_scr[e].rearrange("(t x) -> t x", x=128),
                                  in_=ct_v[:, e, :])
                nc.sync.dma_start(out=c_sb[:, e, :],
                                  in_=c_scr[e:e + 1, :].to_broadcast([128, T]))

            # ---- main pipeline, token-half outer so each half's
            #      ReduceScatter overlaps the other half's compute ----
            h_sb = big.tile([128, E_LOC, NI, T], BF)
            hs_sb = big.tile([128, T], BF)
            shard = T // NCHUNK // NCORES  # 64
            for th in range(2):
                tsl = slice(th * 512, (th + 1) * 512)
                # routed experts: G/U projections + H = silu(G)*U*c
                for e in range(E_LOC):
                    for ni in range(NI):
                        isl = slice(ni * 128, (ni + 1) * 128)
                        g_ps = ps_gu.tile([128, 512], F32, tag="g")
                        for k in range(KT):
                            nc.tensor.matmul(g_ps[:], wg_sb[:, e, k, isl],
                                             xtb_sb[:, k, tsl],
                                             start=(k == 0), stop=(k == KT - 1))
                        u_ps = ps_gu.tile([128, 512], F32, tag="u")
                        for k in range(KT):
                            nc.tensor.matmul(u_ps[:], wu_sb[:, e, k, isl],
                                             xtb_sb[:, k, tsl],
                                             start=(k == 0), stop=(k == KT - 1))
                        gs = gs_pool.tile([128, 512], BF, tag="gs")
                        nc.scalar.activation(gs[:], g_ps[:],
                                             mybir.ActivationFunctionType.Silu)
                        nc.vector.tensor_mul(gs[:], gs[:], u_ps[:])
                        nc.vector.tensor_mul(h_sb[:, e, ni, tsl], gs[:],
                                             c_sb[:, e, tsl])
                # shared expert slice: Hs = silu(gate)*up
                sg_ps = ps_gu.tile([128, 512], F32, tag="g")
                for k in range(KT):
                    nc.tensor.matmul(sg_ps[:], wsgu_sb[:, k, 0:ISH],
                                     xtb_sb[:, k, tsl],
                                     start=(k == 0), stop=(k == KT - 1))
                su_ps = ps_gu.tile([128, 512], F32, tag="u")
                for k in range(KT):
                    nc.tensor.matmul(su_ps[:], wsgu_sb[:, k, ISH:2 * ISH],
                                     xtb_sb[:, k, tsl],
                                     start=(k == 0), stop=(k == KT - 1))
                sgs = gs_pool.tile([128, 512], BF, tag="gs")
                nc.scalar.activation(sgs[:], sg_ps[:],
                                     mybir.ActivationFunctionType.Silu)
                nc.vector.tensor_mul(hs_sb[:, tsl], sgs[:], su_ps[:])

                # down projections, one RS per token-quarter chunk
                chunks_per_th = NCHUNK // 2
                tt_per_chunk = NTT // NCHUNK
                for cq in range(chunks_per_th):
                    chunk = th * chunks_per_th + cq
                    for ti in range(tt_per_chunk):
                        tt = chunk * tt_per_chunk + ti
                        dsl = slice(tt * 128, (tt + 1) * 128)
                        lsl = slice(ti * 128, (ti + 1) * 128)  # within chunk
                        for nh in range(2):
                            hsl = slice(nh * 512, (nh + 1) * 512)
                            acc_ps = ps_acc.tile([128, 512], F32, tag="acc")
                            first = True
                            for e in range(E_LOC):
                                for ni in range(NI):
                                    nc.tensor.matmul(acc_ps[:],
                                                     h_sb[:, e, ni, dsl],
                                                     wd_sb[:, e, ni, hsl],
                                                     start=first, stop=False)
                                    first = False
                            nc.tensor.matmul(acc_ps[:], hs_sb[:, dsl],
                                             wsd_sb[:, hsl],
                                             start=False, stop=True)
                            acc_sb = accs.tile([128, 512], F32, tag="accsb")
                            nc.vector.tensor_copy(acc_sb[:], acc_ps[:])
                            nc.sync.dma_start(out=acc_dram[chunk][lsl, hsl],
                                              in_=acc_sb[:])
                    nc.gpsimd.collective_compute(
                        "ReduceScatter", mybir.AluOpType.add, replica_groups=rg,
                        ins=[acc_dram[chunk].opt()], outs=[rs_out[chunk].opt()])
                    # SWDGE for the final writes: a HWDGE queue slot waiting
                    # on the RS would head-of-line-block later acc writes
                    nc.gpsimd.dma_start(
                        out=o[chunk * shard:(chunk + 1) * shard, :],
                        in_=rs_out[chunk])

    nc.compile()
    return nc


_NC = None


def _get_nc():
    global _NC
    if _NC is None:
        _NC = build_nc()
    return _NC


def _make_in_maps(hidden_states, router_w, w_gate, w_up, w_down,
                  ws_gate_up, ws_down):
    xtf = np.ascontiguousarray(hidden_states.T.astype(np.float32))
    xtb = xtf.astype(BF16)
    maps = []
    for c in range(NCORES):
        own = [2 * c, 2 * c + 1]
        rest = [e for e in range(E) if e not in own]
        perm = own + rest
        rwt_c = np.ascontiguousarray(router_w[perm].T.astype(np.float32))
        gate_sl = ws_gate_up[:, c * ISH:(c + 1) * ISH]
        up_sl = ws_gate_up[:, E // 2 * ISH + c * ISH:E // 2 * ISH + (c + 1) * ISH]
        maps.append({
            "xtf": xtf,
            "xtb": xtb,
            "rwt": rwt_c,
            "wg": np.ascontiguousarray(w_gate[own]).astype(BF16),
            "wu": np.ascontiguousarray(w_up[own]).astype(BF16),
            "wd": np.ascontiguousarray(w_down[own]).astype(BF16),
            "wsgu": np.ascontiguousarray(
                np.concatenate([gate_sl, up_sl], axis=1)).astype(BF16),
            "wsd": np.ascontiguousarray(ws_down[c * ISH:(c + 1) * ISH]).astype(BF16),
        })
    return maps


def _assemble(results):
    shard = T // NCHUNK // NCORES
    out = np.empty((T, H), np.float32)
    for c in range(NCORES):
        oc = results[c]["o"]
        for h in range(NCHUNK):
            lo = h * (T // NCHUNK) + c * shard
            out[lo:lo + shard] = oc[h * shard:(h + 1) * shard]
    return out


def run(inputs, trace=False):
    """Run on hardware; returns (output, exec_time_ns or None)."""
    nc = _get_nc()
    maps = _make_in_maps(**inputs)
    res = run_bass_kernel_spmd(nc, maps, list(range(NCORES)), trace=trace)
    return _assemble(res.results), res.exec_time_ns


def kernel(**inputs):
    out, _ = run(inputs, trace=False)
    return out


# revision 12
# speedup vs baseline: 1.1363x; 1.1363x over previous
"""BailingMoE block (router + 16 routed experts top-4 + shared SwiGLU MLP)
as a Trainium2 Bass/Tile kernel, expert-parallel over 8 NeuronCores.

Sharding:
  - Routed expert weight stacks [E,H,I] split along E: 2 experts per core
    (cast to bf16 on host; fp32 PSUM accumulation on device).
  - Shared-expert MLP tensor-parallel along the intermediate dim: 128 of
    1024 shared-intermediate channels per core.
  - Router replicated (fp32 - top-4 selection must match the reference);
    per-core the router weight columns are permuted so that each core's own
    2 experts land in columns 0/1 (softmax/top-k are permutation invariant).
  - Each core produces a full [T,H] partial (its experts + its shared
    slice); two ReduceScatters (token halves) sum the partials on-device
    and leave each core with a disjoint 2x64-token slice of the output,
    which the host concatenates (pure unshard, no host math).

Device dataflow (per core, all matmuls bf16 with fp32 accumulation):
  Xt = X^T staged [H,T];  G_t/U_t = Wg^T X^T per expert in [I,T] layout so
  both operands of every matmul are in their native layout (no transposes
  on the heavy path);  H = silu(G)*U*combine;  down-proj accumulates both
  experts + shared slice into one PSUM tile per (token-tile, out-half).
"""

import numpy as np
import ml_dtypes

import concourse.bass as bass
import concourse.mybir as mybir
import concourse.tile as tile
from concourse import bacc
from concourse.bass_utils import run_bass_kernel_spmd
from concourse.masks import make_identity

BF16 = ml_dtypes.bfloat16

NCORES = 8
T = 1024
H = 1024
I = 512  # routed expert intermediate
E = 16
TOP_K = 4
E_LOC = 2  # experts per core
ISH = 128  # shared-intermediate channels per core (1024 / 8)
KT = H // 128  # 8 contraction tiles over H
NI = I // 128  # 4 partition tiles over I
NTT = T // 128  # 8 token tiles
NCHUNK = 2  # ReduceScatter chunks (token halves)

F32 = mybir.dt.float32
BF = mybir.dt.bfloat16


def build_nc():
    nc = bacc.Bacc("TRN2", target_bir_lowering=False, debug=False,
                   num_devices=NCORES)

    xtf = nc.dram_tensor("xtf", [H, T], F32, kind="ExternalInput")
    xtb = nc.dram_tensor("xtb", [H, T], BF, kind="ExternalInput")
    rwt = nc.dram_tensor("rwt", [H, E], F32, kind="ExternalInput")
    wg = nc.dram_tensor("wg", [E_LOC, H, I], BF, kind="ExternalInput")
    wu = nc.dram_tensor("wu", [E_LOC, H, I], BF, kind="ExternalInput")
    wd = nc.dram_tensor("wd", [E_LOC, I, H], BF, kind="ExternalInput")
    wsgu = nc.dram_tensor("wsgu", [H, 2 * ISH], BF, kind="ExternalInput")
    wsd = nc.dram_tensor("wsd", [ISH, H], BF, kind="ExternalInput")
    o = nc.dram_tensor("o", [NCHUNK * (T // NCHUNK // NCORES), H], F32,
                       kind="ExternalOutput")

    rg = [list(range(NCORES))]

    with tile.TileContext(nc) as tc:
        with (
            tc.tile_pool(name="big", bufs=1) as big,
            tc.tile_pool(name="small", bufs=3) as small,
            tc.tile_pool(name="gs_pool", bufs=3) as gs_pool,
            tc.tile_pool(name="accs", bufs=3) as accs,
            tc.tile_pool(name="ps_small", bufs=1, space="PSUM") as ps_small,
            tc.tile_pool(name="ps_gu", bufs=2, space="PSUM") as ps_gu,
            tc.tile_pool(name="ps_acc", bufs=2, space="PSUM") as ps_acc,
            tc.tile_pool(name="dram", bufs=1, space="DRAM") as dram,
        ):
            # ---- staged inputs (everything fits in SBUF); DMAs chunked and
            # emitted in consumption order so compute starts early ----
            rwt_sb = big.tile([128, KT, E], F32)
            nc.sync.dma_start(out=rwt_sb, in_=rwt.rearrange("(k p) e -> p k e", p=128))
            xtf_r = xtf.rearrange("(k p) t -> p k t", p=128)
            xtf_sb = big.tile([128, KT, T], F32)
            for k in range(KT):
                nc.sync.dma_start(out=xtf_sb[:, k, :], in_=xtf_r[:, k, :])
            xtb_r = xtb.rearrange("(k p) t -> p k t", p=128)
            xtb_sb = big.tile([128, KT, T], BF)
            for k in range(KT):
                nc.sync.dma_start(out=xtb_sb[:, k, :], in_=xtb_r[:, k, :])
            wg_sb = big.tile([128, E_LOC, KT, I], BF)
            wu_sb = big.tile([128, E_LOC, KT, I], BF)
            wg_r = wg.rearrange("e (k p) i -> p e k i", p=128)
            wu_r = wu.rearrange("e (k p) i -> p e k i", p=128)
            for e in range(E_LOC):
                nc.sync.dma_start(out=wg_sb[:, e], in_=wg_r[:, e])
                nc.sync.dma_start(out=wu_sb[:, e], in_=wu_r[:, e])
            wsgu_sb = big.tile([128, KT, 2 * ISH], BF)
            nc.sync.dma_start(out=wsgu_sb, in_=wsgu.rearrange("(k p) i -> p k i", p=128))
            wd_sb = big.tile([128, E_LOC, NI, H], BF)
            nc.sync.dma_start(out=wd_sb, in_=wd.rearrange("e (n p) h -> p e n h", p=128))
            wsd_sb = big.tile([128, H], BF)
            nc.sync.dma_start(out=wsd_sb, in_=wsd[:])

            identity = big.tile([128, 128], F32)
            make_identity(nc, identity)

            # one DRAM tensor per chunk: a shared tensor would put a false
            # WAR dependency between chunk k's RS read and chunk k+1's writes
            acc_dram = [dram.tile([T // NCHUNK, H], F32, name=f"acc_dram{i}")
                        for i in range(NCHUNK)]
            rs_out = dram.tile([NCHUNK, T // NCHUNK // NCORES, H], F32)
            c_scr = dram.tile([E_LOC, T], BF)

            # tiny collective up front: absorbs the expensive first-collective
            # ncfw init / rank rendezvous while compute streams
            warm_in = dram.tile([128, 1], F32)
            warm_out = dram.tile([128, 1], F32, addr_space="Shared")
            nc.gpsimd.collective_compute(
                "AllReduce", mybir.AluOpType.add, replica_groups=rg,
                ins=[warm_in.opt()], outs=[warm_out.opt()])

            # ---- router: logits -> top-4 -> normalized combine weights.
            # Logits computed transposed ([E,T]: 16 N=512 fp32 matmuls beats
            # 64 N=16 ones), PE-transposed back per token tile; all vector
            # work batched over the 8 token tiles ([128, 8, 16] layout) to
            # avoid serial cross-engine round-trips. ----
            lgt_ps = ps_small.tile([E, T], F32, tag="sm")
            for th in range(2):
                tsl = slice(th * 512, (th + 1) * 512)
                for k in range(KT):
                    nc.tensor.matmul(lgt_ps[:, tsl], rwt_sb[:, k, :],
                                     xtf_sb[:, k, tsl],
                                     start=(k == 0), stop=(k == KT - 1))
            lgt_sb = small.tile([E, T], F32)
            nc.vector.tensor_copy(lgt_sb[:], lgt_ps[:])
            L = small.tile([128, NTT, E], F32)
            for tt in range(NTT):
                tr_ps = ps_acc.tile([128, E], F32, tag="acc")
                nc.tensor.transpose(tr_ps[:], lgt_sb[:, tt * 128:(tt + 1) * 128],
                                    identity[0:E, 0:E])
                nc.vector.tensor_copy(L[:, tt, :], tr_ps[:])
            # 4 rounds of (rowmax, knock out the max) to find the 4th-largest
            m = small.tile([128, NTT, 4], F32)
            msk = small.tile([128, NTT, E], F32)
            Lw = small.tile([128, NTT, E], F32)
            nc.vector.tensor_copy(Lw[:], L[:])
            for r in range(4):
                nc.vector.reduce_max(m[:, :, r], Lw[:], axis=mybir.AxisListType.X)
                if r < 3:
                    nc.vector.tensor_tensor(msk[:], Lw[:],
                                            m[:, :, r:r + 1].to_broadcast([128, NTT, E]),
                                            op=mybir.AluOpType.is_ge)
                    nc.vector.scalar_tensor_tensor(Lw[:], msk[:], -1e30, Lw[:],
                                                   op0=mybir.AluOpType.mult,
                                                   op1=mybir.AluOpType.add)
            # top-4 mask on the original logits; softmax over the masked set
            nc.vector.tensor_tensor(msk[:], L[:],
                                    m[:, :, 3:4].to_broadcast([128, NTT, E]),
                                    op=mybir.AluOpType.is_ge)
            nc.vector.tensor_tensor(L[:], L[:],
                                    m[:, :, 0:1].to_broadcast([128, NTT, E]),
                                    op=mybir.AluOpType.subtract)
            nc.scalar.activation(L[:], L[:], mybir.ActivationFunctionType.Exp)
            nc.vector.tensor_mul(L[:], L[:], msk[:])
            ssum = small.tile([128, NTT, 1], F32)
            nc.vector.reduce_sum(ssum[:, :, 0], L[:], axis=mybir.AxisListType.X)
            rcp = small.tile([128, NTT, 1], F32)
            nc.vector.reciprocal(rcp[:, :, 0], ssum[:, :, 0])
            nc.vector.tensor_mul(L[:], L[:],
                                 rcp[:].to_broadcast([128, NTT, E]))
            # one PE transpose: [tok128, (tt e)] -> [(tt e), tok128]
            ct_ps = ps_small.tile([128, 128], F32, tag="sm")
            nc.tensor.transpose(ct_ps[:], L.rearrange("p t e -> p (t e)"),
                                identity[:])
            ct_sb = small.tile([128, 128], BF)
            nc.vector.tensor_copy(ct_sb[:], ct_ps[:])
            # per-expert combine rows broadcast to 128 partitions via a DRAM
            # round-trip (DMA replicates the row; frees PE/DVE)
            ct_v = ct_sb.rearrange("(t e) x -> t e x", e=E)
            c_sb = big.tile([128, E_LOC, T], BF)
            for e in range(E_LOC):
                nc.sync.dma_start(out=c_scr[e].rearrange("(t x) -> t x", x=128),
                                  in_=ct_v[:, e, :])
                nc.sync.dma_start(out=c_sb[:, e, :],
                                  in_=c_scr[e:e + 1, :].to_broadcast([128, T]))

            # ---- main pipeline, token-half outer so each half's
            #      ReduceScatter overlaps the other half's compute ----
            h_sb = big.tile([128, E_LOC, NI, T], BF)
            hs_sb = big.tile([128, T], BF)
            shard = T // NCHUNK // NCORES  # 64
            for th in range(2):
                tsl = slice(th * 512, (th + 1) * 512)
                # routed experts: G/U projections + H = silu(G)*U*c
                for e in range(E_LOC):
                    for ni in range(NI):
                        isl = slice(ni * 128, (ni + 1) * 128)
                        g_ps = ps_gu.tile([128, 512], F32, tag="g")
                        for k in range(KT):
                            nc.tensor.matmul(g_ps[:], wg_sb[:, e, k, isl],
                                             xtb_sb[:, k, tsl],
                                             start=(k == 0), stop=(k == KT - 1))
                        u_ps = ps_gu.tile([128, 512], F32, tag="u")
                        for k in range(KT):
                            nc.tensor.matmul(u_ps[:], wu_sb[:, e, k, isl],
                                             xtb_sb[:, k, tsl],
                                             start=(k == 0), stop=(k == KT - 1))
                        gs = gs_pool.tile([128, 512], BF, tag="gs")
                        nc.scalar.activation(gs[:], g_ps[:],
                                             mybir.ActivationFunctionType.Silu)
                        nc.vector.tensor_mul(gs[:], gs[:], u_ps[:])
                        nc.vector.tensor_mul(h_sb[:, e, ni, tsl], gs[:],
                                             c_sb[:, e, tsl])
                # shared expert slice: Hs = silu(gate)*up
                sg_ps = ps_gu.tile([128, 512], F32, tag="g")
                for k in range(KT):
                    nc.tensor.matmul(sg_ps[:], wsgu_sb[:, k, 0:ISH],
                                     xtb_sb[:, k, tsl],
                                     start=(k == 0), stop=(k == KT - 1))
                su_ps = ps_gu.tile([128, 512], F32, tag="u")
                for k in range(KT):
                    nc.tensor.matmul(su_ps[:], wsgu_sb[:, k, ISH:2 * ISH],
                                     xtb_sb[:, k, tsl],
                                     start=(k == 0), stop=(k == KT - 1))
                sgs = gs_pool.tile([128, 512], BF, tag="gs")
                nc.scalar.activation(sgs[:], sg_ps[:],
                                     mybir.ActivationFunctionType.Silu)
                nc.vector.tensor_mul(hs_sb[:, tsl], sgs[:], su_ps[:])

                # down projections, one RS per token-quarter chunk
                chunks_per_th = NCHUNK // 2
                tt_per_chunk = NTT // NCHUNK
                for cq in range(chunks_per_th):
                    chunk = th * chunks_per_th + cq
                    for ti in range(tt_per_chunk):
                        tt = chunk * tt_per_chunk + ti
                        dsl = slice(tt * 128, (tt + 1) * 128)
                        lsl = slice(ti * 128, (ti + 1) * 128)  # within chunk
                        for nh in range(2):
                            hsl = slice(nh * 512, (nh + 1) * 512)
                            acc_ps = ps_acc.tile([128, 512], F32, tag="acc")
                            first = True
                            for e in range(E_LOC):
                                for ni in range(NI):
                                    nc.tensor.matmul(acc_ps[:],
                                                     h_sb[:, e, ni, dsl],
                                                     wd_sb[:, e, ni, hsl],
                                                     start=first, stop=False)
                                    first = False
                            nc.tensor.matmul(acc_ps[:], hs_sb[:, dsl],
                                             wsd_sb[:, hsl],
                                             start=False, stop=True)
                            acc_sb = accs.tile([128, 512], F32, tag="accsb")
                            nc.vector.tensor_copy(acc_sb[:], acc_ps[:])
                            nc.sync.dma_start(out=acc_dram[chunk][lsl, hsl],
                                              in_=acc_sb[:])
                    nc.gpsimd.collective_compute(
                        "ReduceScatter", mybir.AluOpType.add, replica_groups=rg,
                        ins=[acc_dram[chunk].opt()], outs=[rs_out[chunk].opt()])
                    # SWDGE for the final writes: a HWDGE queue slot waiting
                    # on the RS would head-of-line-block later acc writes
                    nc.gpsimd.dma_start(
                        out=o[chunk * shard:(chunk + 1) * shard, :],
                        in_=rs_out[chunk])

    nc.compile()
    return nc


_NC = None


def _get_nc():
    global _NC
    if _NC is None:
        _NC = build_nc()
    return _NC


def _make_in_maps(hidden_states, router_w, w_gate, w_up, w_down,
                  ws_gate_up, ws_down):
    xtf = np.ascontiguousarray(hidden_states.T.astype(np.float32))
    xtb = xtf.astype(BF16)
    maps = []
    for c in range(NCORES):
        own = [2 * c, 2 * c + 1]
        rest = [e for e in range(E) if e not in own]
        perm = own + rest
        rwt_c = np.ascontiguousarray(router_w[perm].T.astype(np.float32))
        gate_sl = ws_gate_up[:, c * ISH:(c + 1) * ISH]
        up_sl = ws_gate_up[:, E // 2 * ISH + c * ISH:E // 2 * ISH + (c + 1) * ISH]
        maps.append({
            "xtf": xtf,
            "xtb": xtb,
            "rwt": rwt_c,
            "wg": np.ascontiguousarray(w_gate[own]).astype(BF16),
            "wu": np.ascontiguousarray(w_up[own]).astype(BF16),
            "wd": np.ascontiguousarray(w_down[own]).astype(BF16),
            "wsgu": np.ascontiguousarray(
                np.concatenate([gate_sl, up_sl], axis=1)).astype(BF16),
            "wsd": np.ascontiguousarray(ws_down[c * ISH:(c + 1) * ISH]).astype(BF16),
        })
    return maps


def _assemble(results):
    shard = T // NCHUNK // NCORES
    out = np.empty((T, H), np.float32)
    for c in range(NCORES):
        oc = results[c]["o"]
        for h in range(NCHUNK):
            lo = h * (T // NCHUNK) + c * shard
            out[lo:lo + shard] = oc[h * shard:(h + 1) * shard]
    return out


def run(inputs, trace=False):
    """Run on hardware; returns (output, exec_time_ns or None)."""
    nc = _get_nc()
    maps = _make_in_maps(**inputs)
    res = run_bass_kernel_spmd(nc, maps, list(range(NCORES)), trace=trace)
    return _assemble(res.results), res.exec_time_ns


def kernel(**inputs):
    out, _ = run(inputs, trace=False)
    return out


# revision 13
# speedup vs baseline: 1.1577x; 1.0189x over previous
"""BailingMoE block (router + 16 routed experts top-4 + shared SwiGLU MLP)
as a Trainium2 Bass/Tile kernel, expert-parallel over 8 NeuronCores.

Sharding:
  - Routed expert weight stacks [E,H,I] split along E: 2 experts per core
    (cast to bf16 on host; fp32 PSUM accumulation on device).
  - Shared-expert MLP tensor-parallel along the intermediate dim: 128 of
    1024 shared-intermediate channels per core.
  - Router replicated (fp32 - top-4 selection must match the reference);
    per-core the router weight columns are permuted so that each core's own
    2 experts land in columns 0/1 (softmax/top-k are permutation invariant).
  - Each core produces a full [T,H] partial (its experts + its shared
    slice); two ReduceScatters (token halves) sum the partials on-device
    and leave each core with a disjoint 2x64-token slice of the output,
    which the host concatenates (pure unshard, no host math).

Device dataflow (per core, all matmuls bf16 with fp32 accumulation):
  Xt = X^T staged [H,T];  G_t/U_t = Wg^T X^T per expert in [I,T] layout so
  both operands of every matmul are in their native layout (no transposes
  on the heavy path);  H = silu(G)*U*combine;  down-proj accumulates both
  experts + shared slice into one PSUM tile per (token-tile, out-half).
"""

import numpy as np
import ml_dtypes

import concourse.bass as bass
import concourse.mybir as mybir
import concourse.tile as tile
from concourse import bacc
from concourse.bass_utils import run_bass_kernel_spmd
from concourse.masks import make_identity

BF16 = ml_dtypes.bfloat16

NCORES = 8
T = 1024
H = 1024
I = 512  # routed expert intermediate
E = 16
TOP_K = 4
E_LOC = 2  # experts per core
ISH = 128  # shared-intermediate channels per core (1024 / 8)
KT = H // 128  # 8 contraction tiles over H
NI = I // 128  # 4 partition tiles over I
NTT = T // 128  # 8 token tiles
NCHUNK = 2  # ReduceScatter chunks (token halves)

F32 = mybir.dt.float32
BF = mybir.dt.bfloat16


def build_nc():
    nc = bacc.Bacc("TRN2", target_bir_lowering=False, debug=False,
                   num_devices=NCORES)

    xtf = nc.dram_tensor("xtf", [H, T], F32, kind="ExternalInput")
    xtb = nc.dram_tensor("xtb", [H, T], BF, kind="ExternalInput")
    rwt = nc.dram_tensor("rwt", [H, E], F32, kind="ExternalInput")
    wg = nc.dram_tensor("wg", [E_LOC, H, I], BF, kind="ExternalInput")
    wu = nc.dram_tensor("wu", [E_LOC, H, I], BF, kind="ExternalInput")
    wd = nc.dram_tensor("wd", [E_LOC, I, H], BF, kind="ExternalInput")
    wsgu = nc.dram_tensor("wsgu", [H, 2 * ISH], BF, kind="ExternalInput")
    wsd = nc.dram_tensor("wsd", [ISH, H], BF, kind="ExternalInput")
    o = nc.dram_tensor("o", [NCHUNK * (T // NCHUNK // NCORES), H], BF,
                       kind="ExternalOutput")

    rg = [list(range(NCORES))]

    with tile.TileContext(nc) as tc:
        with (
            tc.tile_pool(name="big", bufs=1) as big,
            tc.tile_pool(name="small", bufs=3) as small,
            tc.tile_pool(name="gs_pool", bufs=3) as gs_pool,
            tc.tile_pool(name="accs", bufs=3) as accs,
            tc.tile_pool(name="ps_small", bufs=1, space="PSUM") as ps_small,
            tc.tile_pool(name="ps_gu", bufs=2, space="PSUM") as ps_gu,
            tc.tile_pool(name="ps_acc", bufs=2, space="PSUM") as ps_acc,
            tc.tile_pool(name="dram", bufs=1, space="DRAM") as dram,
        ):
            # ---- staged inputs (everything fits in SBUF); DMAs chunked and
            # emitted in consumption order so compute starts early ----
            rwt_sb = big.tile([128, KT, E], F32)
            nc.sync.dma_start(out=rwt_sb, in_=rwt.rearrange("(k p) e -> p k e", p=128))
            xtf_r = xtf.rearrange("(k p) t -> p k t", p=128)
            xtf_sb = big.tile([128, KT, T], F32)
            for k in range(KT):
                nc.sync.dma_start(out=xtf_sb[:, k, :], in_=xtf_r[:, k, :])
            xtb_r = xtb.rearrange("(k p) t -> p k t", p=128)
            xtb_sb = big.tile([128, KT, T], BF)
            for k in range(KT):
                nc.sync.dma_start(out=xtb_sb[:, k, :], in_=xtb_r[:, k, :])
            wg_sb = big.tile([128, E_LOC, KT, I], BF)
            wu_sb = big.tile([128, E_LOC, KT, I], BF)
            wg_r = wg.rearrange("e (k p) i -> p e k i", p=128)
            wu_r = wu.rearrange("e (k p) i -> p e k i", p=128)
            for e in range(E_LOC):
                nc.sync.dma_start(out=wg_sb[:, e], in_=wg_r[:, e])
                nc.sync.dma_start(out=wu_sb[:, e], in_=wu_r[:, e])
            wsgu_sb = big.tile([128, KT, 2 * ISH], BF)
            nc.sync.dma_start(out=wsgu_sb, in_=wsgu.rearrange("(k p) i -> p k i", p=128))
            wd_sb = big.tile([128, E_LOC, NI, H], BF)
            nc.sync.dma_start(out=wd_sb, in_=wd.rearrange("e (n p) h -> p e n h", p=128))
            wsd_sb = big.tile([128, H], BF)
            nc.sync.dma_start(out=wsd_sb, in_=wsd[:])

            identity = big.tile([128, 128], F32)
            make_identity(nc, identity)

            # one DRAM tensor per chunk: a shared tensor would put a false
            # WAR dependency between chunk k's RS read and chunk k+1's writes
            acc_dram = [dram.tile([T // NCHUNK, H], BF, name=f"acc_dram{i}")
                        for i in range(NCHUNK)]
            rs_out = dram.tile([NCHUNK, T // NCHUNK // NCORES, H], BF)
            c_scr = dram.tile([E_LOC, T], BF)

            # tiny collective up front: absorbs the expensive first-collective
            # ncfw init / rank rendezvous while compute streams
            warm_in = dram.tile([128, 1], F32)
            warm_out = dram.tile([128, 1], F32, addr_space="Shared")
            nc.gpsimd.collective_compute(
                "AllReduce", mybir.AluOpType.add, replica_groups=rg,
                ins=[warm_in.opt()], outs=[warm_out.opt()])

            # ---- router: logits -> top-4 -> normalized combine weights.
            # Logits computed transposed ([E,T]: 16 N=512 fp32 matmuls beats
            # 64 N=16 ones), PE-transposed back per token tile; all vector
            # work batched over the 8 token tiles ([128, 8, 16] layout) to
            # avoid serial cross-engine round-trips. ----
            lgt_ps = ps_small.tile([E, T], F32, tag="sm")
            for th in range(2):
                tsl = slice(th * 512, (th + 1) * 512)
                for k in range(KT):
                    nc.tensor.matmul(lgt_ps[:, tsl], rwt_sb[:, k, :],
                                     xtf_sb[:, k, tsl],
                                     start=(k == 0), stop=(k == KT - 1))
            lgt_sb = small.tile([E, T], F32)
            nc.vector.tensor_copy(lgt_sb[:], lgt_ps[:])
            L = small.tile([128, NTT, E], F32)
            for tt in range(NTT):
                tr_ps = ps_acc.tile([128, E], F32, tag="acc")
                nc.tensor.transpose(tr_ps[:], lgt_sb[:, tt * 128:(tt + 1) * 128],
                                    identity[0:E, 0:E])
                nc.vector.tensor_copy(L[:, tt, :], tr_ps[:])
            # 4 rounds of (rowmax, knock out the max) to find the 4th-largest
            m = small.tile([128, NTT, 4], F32)
            msk = small.tile([128, NTT, E], F32)
            Lw = small.tile([128, NTT, E], F32)
            nc.vector.tensor_copy(Lw[:], L[:])
            for r in range(4):
                nc.vector.reduce_max(m[:, :, r], Lw[:], axis=mybir.AxisListType.X)
                if r < 3:
                    nc.vector.tensor_tensor(msk[:], Lw[:],
                                            m[:, :, r:r + 1].to_broadcast([128, NTT, E]),
                                            op=mybir.AluOpType.is_ge)
                    nc.vector.scalar_tensor_tensor(Lw[:], msk[:], -1e30, Lw[:],
                                                   op0=mybir.AluOpType.mult,
                                                   op1=mybir.AluOpType.add)
            # top-4 mask on the original logits; softmax over the masked set
            nc.vector.tensor_tensor(msk[:], L[:],
                                    m[:, :, 3:4].to_broadcast([128, NTT, E]),
                                    op=mybir.AluOpType.is_ge)
            nc.vector.tensor_tensor(L[:], L[:],
                                    m[:, :, 0:1].to_broadcast([128, NTT, E]),
                                    op=mybir.AluOpType.subtract)
            nc.scalar.activation(L[:], L[:], mybir.ActivationFunctionType.Exp)
            nc.vector.tensor_mul(L[:], L[:], msk[:])
            ssum = small.tile([128, NTT, 1], F32)
            nc.vector.reduce_sum(ssum[:, :, 0], L[:], axis=mybir.AxisListType.X)
            rcp = small.tile([128, NTT, 1], F32)
            nc.vector.reciprocal(rcp[:, :, 0], ssum[:, :, 0])
            nc.vector.tensor_mul(L[:], L[:],
                                 rcp[:].to_broadcast([128, NTT, E]))
            # one PE transpose: [tok128, (tt e)] -> [(tt e), tok128]
            ct_ps = ps_small.tile([128, 128], F32, tag="sm")
            nc.tensor.transpose(ct_ps[:], L.rearrange("p t e -> p (t e)"),
                                identity[:])
            ct_sb = small.tile([128, 128], BF)
            nc.vector.tensor_copy(ct_sb[:], ct_ps[:])
            # per-expert combine rows broadcast to 128 partitions via a DRAM
            # round-trip (DMA replicates the row; frees PE/DVE)
            ct_v = ct_sb.rearrange("(t e) x -> t e x", e=E)
            c_sb = big.tile([128, E_LOC, T], BF)
            for e in range(E_LOC):
                nc.sync.dma_start(out=c_scr[e].rearrange("(t x) -> t x", x=128),
                                  in_=ct_v[:, e, :])
                nc.sync.dma_start(out=c_sb[:, e, :],
                                  in_=c_scr[e:e + 1, :].to_broadcast([128, T]))

            # ---- main pipeline, token-half outer so each half's
            #      ReduceScatter overlaps the other half's compute ----
            h_sb = big.tile([128, E_LOC, NI, T], BF)
            hs_sb = big.tile([128, T], BF)
            shard = T // NCHUNK // NCORES  # 64
            for th in range(2):
                tsl = slice(th * 512, (th + 1) * 512)
                # routed experts: G/U projections + H = silu(G)*U*c
                for e in range(E_LOC):
                    for ni in range(NI):
                        isl = slice(ni * 128, (ni + 1) * 128)
                        g_ps = ps_gu.tile([128, 512], F32, tag="g")
                        for k in range(KT):
                            nc.tensor.matmul(g_ps[:], wg_sb[:, e, k, isl],
                                             xtb_sb[:, k, tsl],
                                             start=(k == 0), stop=(k == KT - 1))
                        u_ps = ps_gu.tile([128, 512], F32, tag="u")
                        for k in range(KT):
                            nc.tensor.matmul(u_ps[:], wu_sb[:, e, k, isl],
                                             xtb_sb[:, k, tsl],
                                             start=(k == 0), stop=(k == KT - 1))
                        gs = gs_pool.tile([128, 512], BF, tag="gs")
                        nc.scalar.activation(gs[:], g_ps[:],
                                             mybir.ActivationFunctionType.Silu)
                        nc.vector.tensor_mul(gs[:], gs[:], u_ps[:])
                        nc.vector.tensor_mul(h_sb[:, e, ni, tsl], gs[:],
                                             c_sb[:, e, tsl])
                # shared expert slice: Hs = silu(gate)*up
                sg_ps = ps_gu.tile([128, 512], F32, tag="g")
                for k in range(KT):
                    nc.tensor.matmul(sg_ps[:], wsgu_sb[:, k, 0:ISH],
                                     xtb_sb[:, k, tsl],
                                     start=(k == 0), stop=(k == KT - 1))
                su_ps = ps_gu.tile([128, 512], F32, tag="u")
                for k in range(KT):
                    nc.tensor.matmul(su_ps[:], wsgu_sb[:, k, ISH:2 * ISH],
                                     xtb_sb[:, k, tsl],
                                     start=(k == 0), stop=(k == KT - 1))
                sgs = gs_pool.tile([128, 512], BF, tag="gs")
                nc.scalar.activation(sgs[:], sg_ps[:],
                                     mybir.ActivationFunctionType.Silu)
                nc.vector.tensor_mul(hs_sb[:, tsl], sgs[:], su_ps[:])

                # down projections, one RS per token-quarter chunk
                chunks_per_th = NCHUNK // 2
                tt_per_chunk = NTT // NCHUNK
                for cq in range(chunks_per_th):
                    chunk = th * chunks_per_th + cq
                    for ti in range(tt_per_chunk):
                        tt = chunk * tt_per_chunk + ti
                        dsl = slice(tt * 128, (tt + 1) * 128)
                        lsl = slice(ti * 128, (ti + 1) * 128)  # within chunk
                        for nh in range(2):
                            hsl = slice(nh * 512, (nh + 1) * 512)
                            acc_ps = ps_acc.tile([128, 512], F32, tag="acc")
                            first = True
                            for e in range(E_LOC):
                                for ni in range(NI):
                                    nc.tensor.matmul(acc_ps[:],
                                                     h_sb[:, e, ni, dsl],
                                                     wd_sb[:, e, ni, hsl],
                                                     start=first, stop=False)
                                    first = False
                            nc.tensor.matmul(acc_ps[:], hs_sb[:, dsl],
                                             wsd_sb[:, hsl],
                                             start=False, stop=True)
                            acc_sb = accs.tile([128, 512], BF, tag="accsb")
                            nc.vector.tensor_copy(acc_sb[:], acc_ps[:])
                            nc.sync.dma_start(out=acc_dram[chunk][lsl, hsl],
                                              in_=acc_sb[:])
                    nc.gpsimd.collective_compute(
                        "ReduceScatter", mybir.AluOpType.add, replica_groups=rg,
                        ins=[acc_dram[chunk].opt()], outs=[rs_out[chunk].opt()])
                    # SWDGE for the final writes: a HWDGE queue slot waiting
                    # on the RS would head-of-line-block later acc writes
                    nc.gpsimd.dma_start(
                        out=o[chunk * shard:(chunk + 1) * shard, :],
                        in_=rs_out[chunk])

    nc.compile()
    return nc


_NC = None


def _get_nc():
    global _NC
    if _NC is None:
        _NC = build_nc()
    return _NC


def _make_in_maps(hidden_states, router_w, w_gate, w_up, w_down,
                  ws_gate_up, ws_down):
    xtf = np.ascontiguousarray(hidden_states.T.astype(np.float32))
    xtb = xtf.astype(BF16)
    maps = []
    for c in range(NCORES):
        own = [2 * c, 2 * c + 1]
        rest = [e for e in range(E) if e not in own]
        perm = own + rest
        rwt_c = np.ascontiguousarray(router_w[perm].T.astype(np.float32))
        gate_sl = ws_gate_up[:, c * ISH:(c + 1) * ISH]
        up_sl = ws_gate_up[:, E // 2 * ISH + c * ISH:E // 2 * ISH + (c + 1) * ISH]
        maps.append({
            "xtf": xtf,
            "xtb": xtb,
            "rwt": rwt_c,
            "wg": np.ascontiguousarray(w_gate[own]).astype(BF16),
            "wu": np.ascontiguousarray(w_up[own]).astype(BF16),
            "wd": np.ascontiguousarray(w_down[own]).astype(BF16),
            "wsgu": np.ascontiguousarray(
                np.concatenate([gate_sl, up_sl], axis=1)).astype(BF16),
            "wsd": np.ascontiguousarray(ws_down[c * ISH:(c + 1) * ISH]).astype(BF16),
        })
    return maps


def _assemble(results):
    shard = T // NCHUNK // NCORES
    out = np.empty((T, H), np.float32)
    for c in range(NCORES):
        oc = results[c]["o"].astype(np.float32)
        for h in range(NCHUNK):
            lo = h * (T // NCHUNK) + c * shard
            out[lo:lo + shard] = oc[h * shard:(h + 1) * shard]
    return out


def run(inputs, trace=False):
    """Run on hardware; returns (output, exec_time_ns or None)."""
    nc = _get_nc()
    maps = _make_in_maps(**inputs)
    res = run_bass_kernel_spmd(nc, maps, list(range(NCORES)), trace=trace)
    return _assemble(res.results), res.exec_time_ns


def kernel(**inputs):
    out, _ = run(inputs, trace=False)
    return out
